# revision 25
# baseline (speedup 1.0000x reference)
"""Fused multi-head self-attention (degenerate seq-len-1) + LayerNorm for TRN2.

Math: with sequence length 1, softmax over the single key is exactly 1.0, so
attention output == v.  The whole module collapses to

    out = LayerNorm((x @ W_v.T + b_v) @ W_proj.T + b_proj) * gamma + beta
        = LayerNorm(x @ Bm + bias) * gamma + beta

with Bm = (W_proj @ W_v).T and bias = W_proj @ b_v + b_proj (both
batch-independent, folded on the host).  The device kernel is a per-core
[1024,4096]x[4096,4096] matmul (batch data-parallel over 8 cores) fused with
LayerNorm -- computed via one level of Strassen to cut PE work 8->7 block
multiplies (12.5% fewer matmul cycles):

    A = x-shard in 2x2 blocks (A11=rows<512,k<2048, ...), Bm in 2x2 blocks.
    M1=(A11+A22)(B11+B22)  M2=(A21+A22)B11  M3=A11(B12-B22)  M4=A22(B21-B11)
    M5n=(A11+A12)(-B22)    M6=(A21-A11)(B11+B12)  M7=(A12-A22)(B21+B22)
    y11=M1+M4+(M5n+M7)  y12=M3-M5n  y21=M2+M4  y22=M1-M2+M3+M6

All B-side combinations are x-independent -> precomputed on the host (free);
A-side combinations are cheap DVE adds under the PE shadow.  Operands are
fp16 (10-bit mantissa beats bf16; PE rate identical), accumulation in fp32
PSUM, output quadrants accumulate in fp16 SBUF.  Phase order
[M3, M4, M1, M2, M6, M5n&M7] finalizes the bottom row-half two multiply
windows early and interleaves M5n/M7 per chunk in one shared PSUM bank
(a12 -= M5n is read mid-bank, then M7 accumulates on top so a11 gets
-M5+M7 in a single RMW); LayerNorm applies + output DMA all overlap PE work
except the last row-tile's.
"""

import os
import sys

import numpy as np

if "/opt/trn_rl_repo" not in sys.path:
    sys.path.insert(0, "/opt/trn_rl_repo")

P = 128              # SBUF partitions
DIM = 4096
B = 8192
NCORES = 8
BL = B // NCORES     # batch rows per core (1024)
BT = BL // P         # b-tiles per core (8)
BT2 = BT // 2        # b-tiles per Strassen row-half (4)
KO = DIM // P        # contraction tiles total (32)
KO2 = KO // 2        # contraction tiles per half (16)
HN = DIM // 2        # half feature dim (2048)
JC = 256             # matmul free dim
NJC2 = HN // JC      # jc chunks per half (8)
EPS = 1e-5
NWARM_HEAD = 28      # PE warmup matmuls before the first real group
NWARM_FILL = 24      # warmup matmuls interleaved after early groups

_BUILD_CACHE = {}

# Normal phases, in execution order.  Raw-A multiplies (M3, M4) go first:
# no S-combo dependency, so the PE starts as soon as the first x/w pieces
# land, and every S-combo gets a full multiply-window of slack.
#   lhs: ('x', bt_off, ko_off) raw A block in xt layout, or ('s', idx) combo
#   dests: (acc_name, 'copy'|'add'|'sub')  -- first touch of an acc is 'copy'
_MULS = [
    ("M3", ("x", 0, 0), [("a12", "copy"), ("a22", "copy")]),   # A11*(B12-B22)
    ("M4", ("x", BT2, KO2), [("a21", "copy"), ("a11", "copy")]),  # A22*(B21-B11)
    ("M1", ("s", 0), [("a11", "add"), ("a22", "add")]),        # (A11+A22)*(B11+B22)
    ("M2", ("s", 1), [("a21", "add"), ("a22", "sub")]),        # (A21+A22)*B11
    ("M6", ("s", 2), [("a22", "add")]),                        # (A21-A11)*(B11+B12)
]
# phase 5 (chunk stream indices 5 and 6): M5n=(A11+A12)*(-B22) then
# M7=(A12-A22)*(B21+B22) accumulated into the same PSUM bank per chunk.

# S-combos: (op, in0 block, in1 block) in xt layout offsets (bt_off, ko_off)
_SCOMBOS = [
    ("add", (0, 0), (BT2, KO2)),    # S1 = A11+A22   (M1, phase 2)
    ("add", (BT2, 0), (BT2, KO2)),  # S2 = A21+A22   (M2, phase 3)
    ("sub", (BT2, 0), (0, 0)),      # S6 = A21-A11   (M6, phase 4)
    ("add", (0, 0), (0, KO2)),      # S5 = A11+A12   (M5n, phase 5)
    ("sub", (0, KO2), (BT2, KO2)),  # S7 = A12-A22   (M7, phase 5)
]
# S index -> phase that consumes it (build emitted one phase ahead)
_S_USER = {0: 2, 1: 3, 2: 4, 3: 5, 4: 6}

# acc -> (phase finalizing it, stats row-half is top?)
_FINAL_PHASE = {"a21": 3, "a22": 4, "a12": 5, "a11": 5}


def _build(apply_bias: bool, apply_affine: bool):
    key = (apply_bias, apply_affine)
    if key in _BUILD_CACHE:
        return _BUILD_CACHE[key]

    import concourse.mybir as mybir
    import concourse.tile as tile
    from concourse import bacc

    f16 = mybir.dt.float16
    f32 = mybir.dt.float32

    nc = bacc.Bacc("TRN2", target_bir_lowering=False, debug=False,
                   num_devices=NCORES)

    xt_d = nc.declare_dram_parameter("xt", [BT, P, KO, P], f16, isOutput=False)
    # t_d[mi, jc, p, ko, jl] = T_mi[ko*128+p, jc*JC+jl]
    t_d = nc.declare_dram_parameter("t", [7, NJC2, P, KO2, JC], f16,
                                    isOutput=False)
    bias_d = nc.declare_dram_parameter("bias", [DIM], f32, isOutput=False)
    gamma_d = nc.declare_dram_parameter("gamma", [DIM], f32, isOutput=False)
    beta_d = nc.declare_dram_parameter("beta", [DIM], f32, isOutput=False)
    out_d = nc.declare_dram_parameter("out", [BT, P, 2 * NJC2, JC], f16,
                                      isOutput=True)

    with tile.TileContext(nc) as tc:
        with tc.tile_pool(name="xpool", bufs=1) as xpool, \
             tc.tile_pool(name="spool", bufs=2) as spool, \
             tc.tile_pool(name="wpool", bufs=8) as wpool, \
             tc.tile_pool(name="apool", bufs=1) as apool, \
             tc.tile_pool(name="small", bufs=4) as small, \
             tc.tile_pool(name="ppool", bufs=8, space="PSUM") as ppool:

            # --- PE warmup: get the HAM clock gate to 8/8 during the DMA
            # head; more batches are interleaved after the first real groups
            # to bridge DMA-ramp stalls without letting HAM re-throttle.
            warm_sb = small.tile([P, 2 * P], f16, name="warm_sb", tag="warm")
            nc.vector.memset(warm_sb, 0.0)
            warm_ps = ppool.tile([P, JC], f32, name="warm_ps", tag="ps")

            def emit_warm(n):
                for _ in range(n):
                    nc.tensor.matmul(warm_ps[:, 0:P], lhsT=warm_sb[:, 0:P],
                                     rhs=warm_sb[:, P:2 * P],
                                     start=True, stop=True)

            emit_warm(NWARM_HEAD)

            # --- B-combo stream: chunk = [P, KO2, JC] fp16 (1 MiB) streamed
            # as two [P, 8, JC] pieces on two DMA rings; the first chunk is
            # split finer so the first accumulation group starts after
            # ~128 KiB.  Phase 5 interleaves the M5n and M7 streams.
            _CHUNKS = [(mi, jc) for mi in range(5) for jc in range(NJC2)] + \
                      [(mi, jc) for jc in range(NJC2) for mi in (5, 6)]
            w_tiles = {}

            def emit_w_chunk(fi):
                mi, jc = _CHUNKS[fi]
                t0 = wpool.tile([P, 8, JC], f16, name="w_sb", tag="w")
                t1 = wpool.tile([P, 8, JC], f16, name="w_sb", tag="w")
                if fi == 0:
                    for a, b in ((0, 1), (1, 2), (2, 4), (4, 8)):
                        nc.sync.dma_start(out=t0[:, a:b],
                                          in_=t_d[mi, jc, :, a:b])
                else:
                    nc.sync.dma_start(out=t0, in_=t_d[mi, jc, :, 0:8])
                nc.scalar.dma_start(out=t1, in_=t_d[mi, jc, :, 8:16])
                w_tiles[fi] = (t0, t1)

            NPREF = 4
            for fi0 in range(NPREF):
                emit_w_chunk(fi0)

            # --- x stream: ALL on the gpsimd ring (sync carries w-t0,
            # scalar carries w-t1; a third stream on either would delay
            # w pieces).  Emission order == need order: A11 per b-tile
            # (phase 0 consumes btl 0..3 in sequence), then A22 (phase 1),
            # then A21/A12 (S-combos of phases 3+).
            xt_sb = xpool.tile([P, BT, KO, P], f16)

            def emit_x(bt, a, b):
                nc.gpsimd.dma_start(out=xt_sb[:, bt, a:b],
                                    in_=xt_d[bt, :, a:b])

            emit_x(0, 0, 2)
            emit_x(0, 2, 4)
            emit_x(0, 4, 8)
            emit_x(0, 8, 16)
            for bt in range(1, BT2):
                emit_x(bt, 0, 8)
                emit_x(bt, 8, 16)
            for bt in range(BT2, BT):          # A22
                emit_x(bt, KO2, KO2 + 8)
                emit_x(bt, KO2 + 8, KO)
            for bt in range(BT2, BT):          # A21
                emit_x(bt, 0, 16)
            for bt in range(0, BT2):           # A12
                emit_x(bt, KO2, KO)

            # --- S-combo tiles (2 rotating slots; built one phase ahead).
            s_tiles = {}

            def emit_s(si):
                op, (b0, k0), (b1, k1) = _SCOMBOS[si]
                s_sb = spool.tile([P, BT2, KO2, P], f16, name="s_sb", tag="s")
                fn = nc.vector.tensor_add if op == "add" else nc.vector.tensor_sub
                for btl in range(BT2):
                    fn(s_sb[:, btl],
                       xt_sb[:, b0 + btl, k0:k0 + KO2],
                       xt_sb[:, b1 + btl, k1:k1 + KO2])
                s_tiles[si] = s_sb

            emit_s(0)  # S1: build under the two raw-A multiplies

            # --- quadrant accumulators (fp16) + LayerNorm state
            acc = {q: apool.tile([P, BT2, NJC2, JC], f16, name=q, tag=q)
                   for q in ("a11", "a12", "a21", "a22")}
            stats_sb = spool.tile([P, BT, 2 * NJC2, 6], f32, name="stats",
                                  tag="stats")
            eps_sb = small.tile([P, 1], f32, name="eps", tag="eps")
            nc.vector.memset(eps_sb, EPS)

            bias_sb = None
            if apply_bias:
                bias_sb = spool.tile([P, 2 * NJC2, JC], f32, name="bias_sb",
                                     tag="bias")
                nc.sync.dma_start(out=bias_sb,
                                  in_=bias_d.ap().to_broadcast(
                                      [P, 2 * NJC2, JC]))
            gamma_sb = beta_sb = None
            if apply_affine:
                gamma_sb = spool.tile([P, 2 * NJC2, JC], f32, name="gamma_sb",
                                      tag="gamma")
                nc.sync.dma_start(out=gamma_sb,
                                  in_=gamma_d.ap().to_broadcast(
                                      [P, 2 * NJC2, JC]))
                beta_sb = spool.tile([P, 2 * NJC2, JC], f32, name="beta_sb",
                                     tag="beta")
                nc.sync.dma_start(out=beta_sb,
                                  in_=beta_d.ap().to_broadcast(
                                      [P, 2 * NJC2, JC]))

            def finalize_chunk(dst, bt, ci):
                if apply_bias:
                    nc.vector.tensor_add(dst, dst, bias_sb[:, ci])
                nc.vector.bn_stats(stats_sb[:, bt, ci], dst)

            def layernorm_apply(bt):
                """Normalize row-block bt (128 rows x 4096) and DMA it out.

                Rows are partitions, so mean/rstd are per-partition scalars.
                Left half applies on DVE (tensor_scalar, 2x fp16 mode),
                right half on ACT (Identity with per-partition scale/bias),
                so the two halves run concurrently.
                """
                top = bt < BT2
                btl = bt if top else bt - BT2
                accL = acc["a11"] if top else acc["a21"]
                accR = acc["a12"] if top else acc["a22"]
                mv = small.tile([P, 2], f32, name="mv", tag="mv")
                nc.vector.bn_aggr(mv, stats_sb[:, bt])
                std = small.tile([P, 1], f32, name="std", tag="std")
                nc.scalar.activation(std, mv[:, 1:2],
                                     mybir.ActivationFunctionType.Sqrt,
                                     bias=eps_sb)
                rstd = small.tile([P, 1], f32, name="rstd", tag="rstd")
                nc.vector.reciprocal(rstd, std)
                nmr = small.tile([P, 1], f32, name="nmr", tag="nmr")
                nc.vector.tensor_scalar(
                    nmr, mv[:, 0:1], scalar1=rstd, scalar2=-1.0,
                    op0=mybir.AluOpType.mult, op1=mybir.AluOpType.mult,
                )
                # normalize in place (the acc chunks are dead after this)
                # and DMA straight out of the accumulators -- no staging
                # tile, so nothing serializes on an output-buffer slot.
                oL = accL[:, btl]
                nc.vector.tensor_scalar(
                    oL, oL, scalar1=mv[:, 0:1], scalar2=rstd,
                    op0=mybir.AluOpType.subtract, op1=mybir.AluOpType.mult,
                )
                # right half splits across ACT and DVE so the per-row-block
                # apply critical path is ~1.3us instead of ~2.1us.
                oR = accR[:, btl]
                h = NJC2 // 2
                nc.scalar.activation(
                    oR[:, 0:h], oR[:, 0:h],
                    mybir.ActivationFunctionType.Identity,
                    bias=nmr, scale=rstd,
                )
                nc.vector.tensor_scalar(
                    oR[:, h:], oR[:, h:], scalar1=mv[:, 0:1], scalar2=rstd,
                    op0=mybir.AluOpType.subtract, op1=mybir.AluOpType.mult,
                )
                for side, o in ((0, oL), (1, oR)):
                    if apply_affine:
                        g = gamma_sb[:, side * NJC2:(side + 1) * NJC2]
                        bta = beta_sb[:, side * NJC2:(side + 1) * NJC2]
                        nc.vector.tensor_mul(o, o, g)
                        nc.vector.tensor_add(o, o, bta)
                    nc.gpsimd.dma_start(
                        out=out_d[bt, :, side * NJC2:(side + 1) * NJC2],
                        in_=o)

            def mm_group(ps, lhsT, fi, start, stop, skip_check=False):
                w0, w1 = w_tiles[fi]
                for ko in range(KO2):
                    nc.tensor.matmul(
                        ps,
                        lhsT=lhsT(ko),
                        rhs=(w0 if ko < 8 else w1)[:, ko % 8],
                        start=start and (ko == 0),
                        stop=stop and (ko == KO2 - 1),
                        skip_group_check=skip_check,
                    )

            # --- phases 0-4
            fi = 0
            for mi, (mname, lhs, dests) in enumerate(_MULS):
                for si, user in _S_USER.items():
                    if user == mi + 1:
                        emit_s(si)

                if lhs[0] == "x":
                    _, b_off, k_off = lhs
                    def lhsT2(btl, ko, b_off=b_off, k_off=k_off):
                        return xt_sb[:, b_off + btl, k_off + ko]
                else:
                    s_sb = s_tiles.pop(lhs[1])
                    def lhsT2(btl, ko, s_sb=s_sb):
                        return s_sb[:, btl, ko]

                for jc in range(NJC2):
                    if fi + NPREF < len(_CHUNKS):
                        emit_w_chunk(fi + NPREF)
                    if mname == "M6" and jc == NJC2 - 1:
                        # S7 ahead of the bottom applies in the DVE FIFO so
                        # phase 5's first M7 group isn't gated on them (the
                        # build still waits for M6's last read of S6's slot).
                        emit_s(4)
                    for btl in range(BT2):
                        ps = ppool.tile([P, JC], f32, name="ps", tag="ps")
                        mm_group(ps, lambda ko, btl=btl: lhsT2(btl, ko),
                                 fi, True, True)
                        # bridge per-group x/w-arrival stalls while the DMA
                        # rings ramp (first jc pass only)
                        if mi == 0 and jc == 0 and btl < 3:
                            emit_warm(8)
                        for acc_name, mode in dests:
                            a_t = acc[acc_name]
                            dst = a_t[:, btl, jc]
                            if mode == "copy":
                                nc.scalar.activation(
                                    dst, ps,
                                    mybir.ActivationFunctionType.Copy)
                            elif mode == "add":
                                nc.vector.tensor_add(dst, dst, ps)
                            else:
                                nc.vector.tensor_sub(dst, dst, ps)
                            if _FINAL_PHASE[acc_name] == mi:
                                bt = btl if acc_name in ("a11", "a12") \
                                    else BT2 + btl
                                right = acc_name in ("a12", "a22")
                                ci = (NJC2 + jc) if right else jc
                                finalize_chunk(dst, bt, ci)
                                if mname == "M6" and jc == NJC2 - 1:
                                    # bottom row-block final: apply overlaps
                                    # the entire M5n/M7 phase.
                                    layernorm_apply(bt)
                    w_tiles.pop(fi)
                    # keep the PE fed through the DMA-ramp head
                    if mi == 0 and 1 <= jc <= 3:
                        emit_warm(8)
                    fi += 1

            # --- phase 5: M5n and M7 share one PSUM bank per (jc, btl).
            # Emission is software-pipelined (M5n btl+1 between M5n btl's
            # RMW and M7 btl) so the PE never waits on the mid-bank read.
            s5 = s_tiles.pop(3)
            s7 = s_tiles.pop(4)
            for jc in range(NJC2):
                f5 = fi + 2 * jc          # M5n chunk
                f7 = f5 + 1               # M7 chunk
                for df in (NPREF, NPREF + 1):
                    if f5 + df < len(_CHUNKS):
                        emit_w_chunk(f5 + df)
                ps_t = {}
                for k in range(BT2 + 1):
                    if k < BT2:
                        btl = k
                        ps = ppool.tile([P, JC], f32, name="ps", tag="ps")
                        ps_t[btl] = ps
                        mm_group(ps, lambda ko, btl=btl: s5[:, btl, ko],
                                 f5, True, True)
                        dst = acc["a12"][:, btl, jc]
                        nc.vector.tensor_sub(dst, dst, ps)
                        finalize_chunk(dst, btl, NJC2 + jc)
                    if k >= 1:
                        btl = k - 1
                        ps = ps_t.pop(btl)
                        # accumulate M7 on top of M5n (has_written bits stay
                        # set from the completed M5n group -> PE adds).
                        mm_group(ps, lambda ko, btl=btl: s7[:, btl, ko],
                                 f7, False, True, skip_check=True)
                        dst = acc["a11"][:, btl, jc]
                        nc.vector.tensor_add(dst, dst, ps)
                        finalize_chunk(dst, btl, jc)
                        if jc == NJC2 - 1:
                            # top row-block final: applies stagger per btl
                            # through the last jc pass.
                            layernorm_apply(btl)
                w_tiles.pop(f5)
                w_tiles.pop(f7)

    nc.compile()
    _BUILD_CACHE[key] = nc
    return nc


def kernel(x, W_qkv, b_qkv, W_proj, b_proj, gamma, beta):
    from concourse.bass_utils import run_bass_kernel_spmd

    x = np.asarray(x, dtype=np.float32)
    W_qkv = np.asarray(W_qkv, dtype=np.float32)
    b_qkv = np.asarray(b_qkv, dtype=np.float32)
    W_proj = np.asarray(W_proj, dtype=np.float32)
    b_proj = np.asarray(b_proj, dtype=np.float32)
    gamma = np.asarray(gamma, dtype=np.float32)
    beta = np.asarray(beta, dtype=np.float32)

    # Fold the two projections (q/k are dead: seq len 1 => attention == v).
    W_v = W_qkv[2 * DIM:3 * DIM, :]
    Bm = np.ascontiguousarray((W_proj @ W_v).T)   # [k, n]
    bias_total = W_proj @ b_qkv[2 * DIM:] + b_proj

    B11 = Bm[:HN, :HN]
    B12 = Bm[:HN, HN:]
    B21 = Bm[HN:, :HN]
    B22 = Bm[HN:, HN:]
    # B-combos in phase order: M3, M4, M1, M2, M6, M5n, M7
    Ts = [B12 - B22, B21 - B11, B11 + B22, B11, B11 + B12, -B22, B21 + B22]

    def tile_t(T):
        # [jc, p, ko, jl] = T[ko*128+p, jc*JC+jl]
        return np.ascontiguousarray(
            T.reshape(KO2, P, NJC2, JC).transpose(2, 1, 0, 3)
        ).astype(np.float16)

    t_host = np.stack([tile_t(T) for T in Ts])

    apply_bias = bool(np.any(bias_total))
    apply_affine = not (np.all(gamma == 1.0) and np.all(beta == 0.0))

    nc = _build(apply_bias, apply_affine)

    in_maps = []
    for i in range(NCORES):
        xs = x[i * BL:(i + 1) * BL]           # [BL, DIM]
        # xt[bt, p, ko, b'] = xs[bt*P + b', ko*P + p]
        xt = np.ascontiguousarray(
            xs.T.reshape(KO, P, BT, P).transpose(2, 1, 0, 3)
        ).astype(np.float16)
        in_maps.append({
            "xt": xt,
            "t": t_host,
            "bias": bias_total,
            "gamma": gamma,
            "beta": beta,
        })

    trace = bool(int(os.environ.get("KERNEL_TRACE", "0")))
    res = run_bass_kernel_spmd(nc, in_maps, core_ids=list(range(NCORES)),
                               trace=trace)
    if trace:
        kernel.last_exec_time_ns = res.exec_time_ns
        kernel.last_results = res

    out = np.concatenate(
        [r["out"].reshape(BL, DIM).astype(np.float32) for r in res.results],
        axis=0,
    )
    return out


# revision 26
# speedup vs baseline: 1.0124x; 1.0124x over previous
"""Fused multi-head self-attention (degenerate seq-len-1) + LayerNorm for TRN2.

Math: with sequence length 1, softmax over the single key is exactly 1.0, so
attention output == v.  The whole module collapses to

    out = LayerNorm((x @ W_v.T + b_v) @ W_proj.T + b_proj) * gamma + beta
        = LayerNorm(x @ Bm + bias) * gamma + beta

with Bm = (W_proj @ W_v).T and bias = W_proj @ b_v + b_proj (both
batch-independent, folded on the host).  The device kernel is a per-core
[1024,4096]x[4096,4096] matmul (batch data-parallel over 8 cores) fused with
LayerNorm -- computed via one level of Strassen to cut PE work 8->7 block
multiplies (12.5% fewer matmul cycles):

    A = x-shard in 2x2 blocks (A11=rows<512,k<2048, ...), Bm in 2x2 blocks.
    M1=(A11+A22)(B11+B22)  M2=(A21+A22)B11  M3=A11(B12-B22)  M4=A22(B21-B11)
    M5n=(A11+A12)(-B22)    M6=(A21-A11)(B11+B12)  M7=(A12-A22)(B21+B22)
    y11=M1+M4+(M5n+M7)  y12=M3-M5n  y21=M2+M4  y22=M1-M2+M3+M6

All B-side combinations are x-independent -> precomputed on the host (free);
A-side combinations are cheap DVE adds under the PE shadow.  Operands are
fp16 (10-bit mantissa beats bf16; PE rate identical), accumulation in fp32
PSUM, output quadrants accumulate in fp16 SBUF.  Phase order
[M3, M4, M1, M2, M6, M5n&M7] finalizes the bottom row-half two multiply
windows early and interleaves M5n/M7 per chunk in one shared PSUM bank
(a12 -= M5n is read mid-bank, then M7 accumulates on top so a11 gets
-M5+M7 in a single RMW); LayerNorm applies + output DMA all overlap PE work
except the last row-tile's.
"""

import os
import sys

import numpy as np

if "/opt/trn_rl_repo" not in sys.path:
    sys.path.insert(0, "/opt/trn_rl_repo")

P = 128              # SBUF partitions
DIM = 4096
B = 8192
NCORES = 8
BL = B // NCORES     # batch rows per core (1024)
BT = BL // P         # b-tiles per core (8)
BT2 = BT // 2        # b-tiles per Strassen row-half (4)
KO = DIM // P        # contraction tiles total (32)
KO2 = KO // 2        # contraction tiles per half (16)
HN = DIM // 2        # half feature dim (2048)
JC = 256             # matmul free dim
NJC2 = HN // JC      # jc chunks per half (8)
EPS = 1e-5
NWARM_HEAD = 28      # PE warmup matmuls before the first real group
NWARM_FILL = 24      # warmup matmuls interleaved after early groups

_BUILD_CACHE = {}

# Normal phases, in execution order.  Raw-A multiplies (M3, M4) go first:
# no S-combo dependency, so the PE starts as soon as the first x/w pieces
# land, and every S-combo gets a full multiply-window of slack.
#   lhs: ('x', bt_off, ko_off) raw A block in xt layout, or ('s', idx) combo
#   dests: (acc_name, 'copy'|'add'|'sub')  -- first touch of an acc is 'copy'
_MULS = [
    ("M3", ("x", 0, 0), [("a12", "copy"), ("a22", "copy")]),   # A11*(B12-B22)
    ("M4", ("x", BT2, KO2), [("a21", "copy"), ("a11", "copy")]),  # A22*(B21-B11)
    ("M1", ("s", 0), [("a11", "add"), ("a22", "add")]),        # (A11+A22)*(B11+B22)
    ("M2", ("s", 1), [("a21", "add"), ("a22", "sub")]),        # (A21+A22)*B11
    ("M6", ("s", 2), [("a22", "add")]),                        # (A21-A11)*(B11+B12)
]
# phase 5 (chunk stream indices 5 and 6): M5n=(A11+A12)*(-B22) then
# M7=(A12-A22)*(B21+B22) accumulated into the same PSUM bank per chunk.

# S-combos: (op, in0 block, in1 block) in xt layout offsets (bt_off, ko_off)
_SCOMBOS = [
    ("add", (0, 0), (BT2, KO2)),    # S1 = A11+A22   (M1, phase 2)
    ("add", (BT2, 0), (BT2, KO2)),  # S2 = A21+A22   (M2, phase 3)
    ("sub", (BT2, 0), (0, 0)),      # S6 = A21-A11   (M6, phase 4)
    ("add", (0, 0), (0, KO2)),      # S5 = A11+A12   (M5n, phase 5)
    ("sub", (0, KO2), (BT2, KO2)),  # S7 = A12-A22   (M7, phase 5)
]
# S index -> phase that consumes it (build emitted one phase ahead)
_S_USER = {0: 2, 1: 3, 2: 4, 3: 5, 4: 6}

# acc -> (phase finalizing it, stats row-half is top?)
_FINAL_PHASE = {"a21": 3, "a22": 4, "a12": 5, "a11": 5}


def _build(apply_bias: bool, apply_affine: bool):
    key = (apply_bias, apply_affine)
    if key in _BUILD_CACHE:
        return _BUILD_CACHE[key]

    import concourse.mybir as mybir
    import concourse.tile as tile
    from concourse import bacc

    f16 = mybir.dt.float16
    f32 = mybir.dt.float32

    nc = bacc.Bacc("TRN2", target_bir_lowering=False, debug=False,
                   num_devices=NCORES)

    xt_d = nc.declare_dram_parameter("xt", [BT, P, KO, P], f16, isOutput=False)
    # t_d[mi, jc, p, ko, jl] = T_mi[ko*128+p, jc*JC+jl]
    t_d = nc.declare_dram_parameter("t", [7, NJC2, P, KO2, JC], f16,
                                    isOutput=False)
    bias_d = nc.declare_dram_parameter("bias", [DIM], f32, isOutput=False)
    gamma_d = nc.declare_dram_parameter("gamma", [DIM], f32, isOutput=False)
    beta_d = nc.declare_dram_parameter("beta", [DIM], f32, isOutput=False)
    out_d = nc.declare_dram_parameter("out", [BT, P, 2 * NJC2, JC], f16,
                                      isOutput=True)

    with tile.TileContext(nc) as tc:
        with tc.tile_pool(name="xpool", bufs=1) as xpool, \
             tc.tile_pool(name="spool", bufs=2) as spool, \
             tc.tile_pool(name="wpool", bufs=8) as wpool, \
             tc.tile_pool(name="apool", bufs=1) as apool, \
             tc.tile_pool(name="small", bufs=4) as small, \
             tc.tile_pool(name="ppool", bufs=7, space="PSUM") as ppool:

            # --- PE warmup: get the HAM clock gate to 8/8 during the DMA
            # head; more batches are interleaved after the first real groups
            # to bridge DMA-ramp stalls without letting HAM re-throttle.
            warm_sb = small.tile([P, 2 * P], f16, name="warm_sb", tag="warm")
            nc.vector.memset(warm_sb, 0.0)
            warm_ps = ppool.tile([P, JC], f32, name="warm_ps", tag="ps")

            def emit_warm(n):
                for _ in range(n):
                    nc.tensor.matmul(warm_ps[:, 0:P], lhsT=warm_sb[:, 0:P],
                                     rhs=warm_sb[:, P:2 * P],
                                     start=True, stop=True)

            emit_warm(NWARM_HEAD)

            # --- B-combo stream: chunk = [P, KO2, JC] fp16 (1 MiB) streamed
            # as two [P, 8, JC] pieces on two DMA rings; the first chunk is
            # split finer so the first accumulation group starts after
            # ~128 KiB.  Phase 5 interleaves the M5n and M7 streams.
            _CHUNKS = [(mi, jc) for mi in range(5) for jc in range(NJC2)] + \
                      [(mi, jc) for jc in range(NJC2) for mi in (5, 6)]
            w_tiles = {}

            def emit_w_chunk(fi):
                mi, jc = _CHUNKS[fi]
                t0 = wpool.tile([P, 8, JC], f16, name="w_sb", tag="w")
                t1 = wpool.tile([P, 8, JC], f16, name="w_sb", tag="w")
                if fi == 0:
                    for a, b in ((0, 1), (1, 2), (2, 4), (4, 8)):
                        nc.sync.dma_start(out=t0[:, a:b],
                                          in_=t_d[mi, jc, :, a:b])
                else:
                    nc.sync.dma_start(out=t0, in_=t_d[mi, jc, :, 0:8])
                nc.scalar.dma_start(out=t1, in_=t_d[mi, jc, :, 8:16])
                w_tiles[fi] = (t0, t1)

            NPREF = 4
            for fi0 in range(NPREF):
                emit_w_chunk(fi0)

            # --- x stream: ALL on the gpsimd ring (sync carries w-t0,
            # scalar carries w-t1; a third stream on either would delay
            # w pieces).  Emission order == need order: A11 per b-tile
            # (phase 0 consumes btl 0..3 in sequence), then A22 (phase 1),
            # then A21/A12 (S-combos of phases 3+).
            xt_sb = xpool.tile([P, BT, KO, P], f16)

            def emit_x(bt, a, b):
                nc.gpsimd.dma_start(out=xt_sb[:, bt, a:b],
                                    in_=xt_d[bt, :, a:b])

            emit_x(0, 0, 2)
            emit_x(0, 2, 4)
            emit_x(0, 4, 8)
            emit_x(0, 8, 16)
            for bt in range(1, BT2):
                emit_x(bt, 0, 8)
                emit_x(bt, 8, 16)
            for bt in range(BT2, BT):          # A22
                emit_x(bt, KO2, KO2 + 8)
                emit_x(bt, KO2 + 8, KO)
            for bt in range(BT2, BT):          # A21
                emit_x(bt, 0, 16)
            for bt in range(0, BT2):           # A12
                emit_x(bt, KO2, KO)

            # --- S-combo tiles (2 rotating slots; built one phase ahead).
            s_tiles = {}

            def emit_s(si):
                op, (b0, k0), (b1, k1) = _SCOMBOS[si]
                s_sb = spool.tile([P, BT2, KO2, P], f16, name="s_sb", tag="s")
                fn = nc.vector.tensor_add if op == "add" else nc.vector.tensor_sub
                for btl in range(BT2):
                    fn(s_sb[:, btl],
                       xt_sb[:, b0 + btl, k0:k0 + KO2],
                       xt_sb[:, b1 + btl, k1:k1 + KO2])
                s_tiles[si] = s_sb

            emit_s(0)  # S1: build under the two raw-A multiplies

            # --- quadrant accumulators (fp16) + LayerNorm state
            acc = {q: apool.tile([P, BT2, NJC2, JC], f16, name=q, tag=q)
                   for q in ("a11", "a12", "a21", "a22")}
            stats_sb = spool.tile([P, BT, 2 * NJC2, 6], f32, name="stats",
                                  tag="stats")
            eps_sb = small.tile([P, 1], f32, name="eps", tag="eps")
            nc.vector.memset(eps_sb, EPS)

            bias_sb = None
            if apply_bias:
                bias_sb = spool.tile([P, 2 * NJC2, JC], f32, name="bias_sb",
                                     tag="bias")
                nc.sync.dma_start(out=bias_sb,
                                  in_=bias_d.ap().to_broadcast(
                                      [P, 2 * NJC2, JC]))
            gamma_sb = beta_sb = None
            if apply_affine:
                gamma_sb = spool.tile([P, 2 * NJC2, JC], f32, name="gamma_sb",
                                      tag="gamma")
                nc.sync.dma_start(out=gamma_sb,
                                  in_=gamma_d.ap().to_broadcast(
                                      [P, 2 * NJC2, JC]))
                beta_sb = spool.tile([P, 2 * NJC2, JC], f32, name="beta_sb",
                                     tag="beta")
                nc.sync.dma_start(out=beta_sb,
                                  in_=beta_d.ap().to_broadcast(
                                      [P, 2 * NJC2, JC]))

            def finalize_chunk(dst, bt, ci):
                if apply_bias:
                    nc.vector.tensor_add(dst, dst, bias_sb[:, ci])
                nc.vector.bn_stats(stats_sb[:, bt, ci], dst)

            def layernorm_apply(bt):
                """Normalize row-block bt (128 rows x 4096) and DMA it out.

                Rows are partitions, so mean/rstd are per-partition scalars.
                Left half applies on DVE (tensor_scalar, 2x fp16 mode),
                right half on ACT (Identity with per-partition scale/bias),
                so the two halves run concurrently.
                """
                top = bt < BT2
                btl = bt if top else bt - BT2
                accL = acc["a11"] if top else acc["a21"]
                accR = acc["a12"] if top else acc["a22"]
                mv = small.tile([P, 2], f32, name="mv", tag="mv")
                nc.vector.bn_aggr(mv, stats_sb[:, bt])
                std = small.tile([P, 1], f32, name="std", tag="std")
                nc.scalar.activation(std, mv[:, 1:2],
                                     mybir.ActivationFunctionType.Sqrt,
                                     bias=eps_sb)
                rstd = small.tile([P, 1], f32, name="rstd", tag="rstd")
                nc.vector.reciprocal(rstd, std)
                nmr = small.tile([P, 1], f32, name="nmr", tag="nmr")
                nc.vector.tensor_scalar(
                    nmr, mv[:, 0:1], scalar1=rstd, scalar2=-1.0,
                    op0=mybir.AluOpType.mult, op1=mybir.AluOpType.mult,
                )
                # normalize in place (the acc chunks are dead after this)
                # and DMA straight out of the accumulators -- no staging
                # tile, so nothing serializes on an output-buffer slot.
                oL = accL[:, btl]
                nc.vector.tensor_scalar(
                    oL, oL, scalar1=mv[:, 0:1], scalar2=rstd,
                    op0=mybir.AluOpType.subtract, op1=mybir.AluOpType.mult,
                )
                # right half splits across ACT and DVE so the per-row-block
                # apply critical path is ~1.3us instead of ~2.1us.
                oR = accR[:, btl]
                h = NJC2 // 2
                nc.scalar.activation(
                    oR[:, 0:h], oR[:, 0:h],
                    mybir.ActivationFunctionType.Identity,
                    bias=nmr, scale=rstd,
                )
                nc.vector.tensor_scalar(
                    oR[:, h:], oR[:, h:], scalar1=mv[:, 0:1], scalar2=rstd,
                    op0=mybir.AluOpType.subtract, op1=mybir.AluOpType.mult,
                )
                for side, o in ((0, oL), (1, oR)):
                    if apply_affine:
                        g = gamma_sb[:, side * NJC2:(side + 1) * NJC2]
                        bta = beta_sb[:, side * NJC2:(side + 1) * NJC2]
                        nc.vector.tensor_mul(o, o, g)
                        nc.vector.tensor_add(o, o, bta)
                    nc.gpsimd.dma_start(
                        out=out_d[bt, :, side * NJC2:(side + 1) * NJC2],
                        in_=o)

            def mm_group(ps, lhsT, fi, start, stop, skip_check=False):
                w0, w1 = w_tiles[fi]
                for ko in range(KO2):
                    nc.tensor.matmul(
                        ps,
                        lhsT=lhsT(ko),
                        rhs=(w0 if ko < 8 else w1)[:, ko % 8],
                        start=start and (ko == 0),
                        stop=stop and (ko == KO2 - 1),
                        skip_group_check=skip_check,
                    )

            # --- phases 0-4
            fi = 0
            for mi, (mname, lhs, dests) in enumerate(_MULS):
                for si, user in _S_USER.items():
                    if user == mi + 1:
                        emit_s(si)

                if lhs[0] == "x":
                    _, b_off, k_off = lhs
                    def lhsT2(btl, ko, b_off=b_off, k_off=k_off):
                        return xt_sb[:, b_off + btl, k_off + ko]
                else:
                    s_sb = s_tiles.pop(lhs[1])
                    def lhsT2(btl, ko, s_sb=s_sb):
                        return s_sb[:, btl, ko]

                for jc in range(NJC2):
                    if fi + NPREF < len(_CHUNKS):
                        emit_w_chunk(fi + NPREF)
                    if mname == "M6" and jc == NJC2 - 1:
                        # S7 ahead of the bottom applies in the DVE FIFO so
                        # phase 5's first M7 group isn't gated on them (the
                        # build still waits for M6's last read of S6's slot).
                        emit_s(4)
                    for btl in range(BT2):
                        ps = ppool.tile([P, JC], f32, name="ps", tag="ps")
                        mm_group(ps, lambda ko, btl=btl: lhsT2(btl, ko),
                                 fi, True, True)
                        # bridge per-group x/w-arrival stalls while the DMA
                        # rings ramp (first jc pass only)
                        if mi == 0 and jc == 0 and btl < 3:
                            emit_warm(8)
                        for acc_name, mode in dests:
                            a_t = acc[acc_name]
                            dst = a_t[:, btl, jc]
                            if mode == "copy":
                                nc.scalar.activation(
                                    dst, ps,
                                    mybir.ActivationFunctionType.Copy)
                            elif mode == "add":
                                nc.vector.tensor_add(dst, dst, ps)
                            else:
                                nc.vector.tensor_sub(dst, dst, ps)
                            if _FINAL_PHASE[acc_name] == mi:
                                bt = btl if acc_name in ("a11", "a12") \
                                    else BT2 + btl
                                right = acc_name in ("a12", "a22")
                                ci = (NJC2 + jc) if right else jc
                                finalize_chunk(dst, bt, ci)
                                if mname == "M6" and jc == NJC2 - 1:
                                    # bottom row-block final: apply overlaps
                                    # the entire M5n/M7 phase.
                                    layernorm_apply(bt)
                    w_tiles.pop(fi)
                    # keep the PE fed through the DMA-ramp head
                    if mi == 0 and 1 <= jc <= 3:
                        emit_warm(8)
                    fi += 1

            # --- phase 5: M5n and M7 share one PSUM bank per (jc, btl).
            # Emission is software-pipelined (M5n btl+1 between M5n btl's
            # RMW and M7 btl) so the PE never waits on the mid-bank read.
            s5 = s_tiles.pop(3)
            s7 = s_tiles.pop(4)
            for jc in range(NJC2):
                f5 = fi + 2 * jc          # M5n chunk
                f7 = f5 + 1               # M7 chunk
                for df in (NPREF, NPREF + 1):
                    if f5 + df < len(_CHUNKS):
                        emit_w_chunk(f5 + df)
                ps_t = {}
                for k in range(BT2 + 1):
                    if k < BT2:
                        btl = k
                        ps = ppool.tile([P, JC], f32, name="ps", tag="ps")
                        ps_t[btl] = ps
                        mm_group(ps, lambda ko, btl=btl: s5[:, btl, ko],
                                 f5, True, True)
                        dst = acc["a12"][:, btl, jc]
                        nc.vector.tensor_sub(dst, dst, ps)
                        finalize_chunk(dst, btl, NJC2 + jc)
                    if k >= 1:
                        btl = k - 1
                        ps = ps_t.pop(btl)
                        # accumulate M7 on top of M5n (has_written bits stay
                        # set from the completed M5n group -> PE adds).
                        mm_group(ps, lambda ko, btl=btl: s7[:, btl, ko],
                                 f7, False, True, skip_check=True)
                        dst = acc["a11"][:, btl, jc]
                        nc.vector.tensor_add(dst, dst, ps)
                        finalize_chunk(dst, btl, jc)
                        if jc == NJC2 - 1:
                            # top row-block final: applies stagger per btl
                            # through the last jc pass.
                            layernorm_apply(btl)
                w_tiles.pop(f5)
                w_tiles.pop(f7)

    nc.compile()
    _BUILD_CACHE[key] = nc
    return nc


def kernel(x, W_qkv, b_qkv, W_proj, b_proj, gamma, beta):
    from concourse.bass_utils import run_bass_kernel_spmd

    x = np.asarray(x, dtype=np.float32)
    W_qkv = np.asarray(W_qkv, dtype=np.float32)
    b_qkv = np.asarray(b_qkv, dtype=np.float32)
    W_proj = np.asarray(W_proj, dtype=np.float32)
    b_proj = np.asarray(b_proj, dtype=np.float32)
    gamma = np.asarray(gamma, dtype=np.float32)
    beta = np.asarray(beta, dtype=np.float32)

    # Fold the two projections (q/k are dead: seq len 1 => attention == v).
    W_v = W_qkv[2 * DIM:3 * DIM, :]
    Bm = np.ascontiguousarray((W_proj @ W_v).T)   # [k, n]
    bias_total = W_proj @ b_qkv[2 * DIM:] + b_proj

    B11 = Bm[:HN, :HN]
    B12 = Bm[:HN, HN:]
    B21 = Bm[HN:, :HN]
    B22 = Bm[HN:, HN:]
    # B-combos in phase order: M3, M4, M1, M2, M6, M5n, M7
    Ts = [B12 - B22, B21 - B11, B11 + B22, B11, B11 + B12, -B22, B21 + B22]

    def tile_t(T):
        # [jc, p, ko, jl] = T[ko*128+p, jc*JC+jl]
        return np.ascontiguousarray(
            T.reshape(KO2, P, NJC2, JC).transpose(2, 1, 0, 3)
        ).astype(np.float16)

    t_host = np.stack([tile_t(T) for T in Ts])

    apply_bias = bool(np.any(bias_total))
    apply_affine = not (np.all(gamma == 1.0) and np.all(beta == 0.0))

    nc = _build(apply_bias, apply_affine)

    in_maps = []
    for i in range(NCORES):
        xs = x[i * BL:(i + 1) * BL]           # [BL, DIM]
        # xt[bt, p, ko, b'] = xs[bt*P + b', ko*P + p]
        xt = np.ascontiguousarray(
            xs.T.reshape(KO, P, BT, P).transpose(2, 1, 0, 3)
        ).astype(np.float16)
        in_maps.append({
            "xt": xt,
            "t": t_host,
            "bias": bias_total,
            "gamma": gamma,
            "beta": beta,
        })

    trace = bool(int(os.environ.get("KERNEL_TRACE", "0")))
    res = run_bass_kernel_spmd(nc, in_maps, core_ids=list(range(NCORES)),
                               trace=trace)
    if trace:
        kernel.last_exec_time_ns = res.exec_time_ns
        kernel.last_results = res

    out = np.concatenate(
        [r["out"].reshape(BL, DIM).astype(np.float32) for r in res.results],
        axis=0,
    )
    return out


# revision 30
# speedup vs baseline: 1.0182x; 1.0058x over previous
"""Fused multi-head self-attention (degenerate seq-len-1) + LayerNorm for TRN2.

Math: with sequence length 1, softmax over the single key is exactly 1.0, so
attention output == v.  The whole module collapses to

    out = LayerNorm((x @ W_v.T + b_v) @ W_proj.T + b_proj) * gamma + beta
        = LayerNorm(x @ Bm + bias) * gamma + beta

with Bm = (W_proj @ W_v).T and bias = W_proj @ b_v + b_proj (both
batch-independent, folded on the host).  The device kernel is a per-core
[1024,4096]x[4096,4096] matmul (batch data-parallel over 8 cores) fused with
LayerNorm -- computed via one level of Strassen to cut PE work 8->7 block
multiplies (12.5% fewer matmul cycles):

    A = x-shard in 2x2 blocks (A11=rows<512,k<2048, ...), Bm in 2x2 blocks.
    M1=(A11+A22)(B11+B22)  M2=(A21+A22)B11  M3=A11(B12-B22)  M4=A22(B21-B11)
    M5n=(A11+A12)(-B22)    M6=(A21-A11)(B11+B12)  M7=(A12-A22)(B21+B22)
    y11=M1+M4+(M5n+M7)  y12=M3-M5n  y21=M2+M4  y22=M1-M2+M3+M6

All B-side combinations are x-independent -> precomputed on the host (free);
A-side combinations are cheap DVE adds under the PE shadow.  Operands are
fp16 (10-bit mantissa beats bf16; PE rate identical), accumulation in fp32
PSUM, output quadrants accumulate in fp16 SBUF.  Phase order
[M3, M4, M1, M2, M6, M5n&M7] finalizes the bottom row-half two multiply
windows early and interleaves M5n/M7 per chunk in one shared PSUM bank
(a12 -= M5n is read mid-bank, then M7 accumulates on top so a11 gets
-M5+M7 in a single RMW); LayerNorm applies + output DMA all overlap PE work
except the last row-tile's.
"""

import os
import sys

import numpy as np

if "/opt/trn_rl_repo" not in sys.path:
    sys.path.insert(0, "/opt/trn_rl_repo")

P = 128              # SBUF partitions
DIM = 4096
B = 8192
NCORES = 8
BL = B // NCORES     # batch rows per core (1024)
BT = BL // P         # b-tiles per core (8)
BT2 = BT // 2        # b-tiles per Strassen row-half (4)
KO = DIM // P        # contraction tiles total (32)
KO2 = KO // 2        # contraction tiles per half (16)
HN = DIM // 2        # half feature dim (2048)
JC = 256             # matmul free dim
NJC2 = HN // JC      # jc chunks per half (8)
EPS = 1e-5
NWARM_HEAD = 28      # PE warmup matmuls before the first real group
NWARM_FILL = 24      # warmup matmuls interleaved after early groups

_BUILD_CACHE = {}

# Normal phases, in execution order.  Raw-A multiplies (M3, M4) go first:
# no S-combo dependency, so the PE starts as soon as the first x/w pieces
# land, and every S-combo gets a full multiply-window of slack.
#   lhs: ('x', bt_off, ko_off) raw A block in xt layout, or ('s', idx) combo
#   dests: (acc_name, 'copy'|'add'|'sub')  -- first touch of an acc is 'copy'
_MULS = [
    ("M3", ("x", 0, 0), [("a12", "copy"), ("a22", "copy")]),   # A11*(B12-B22)
    ("M4", ("x", BT2, KO2), [("a21", "copy"), ("a11", "copy")]),  # A22*(B21-B11)
    ("M1", ("s", 0), [("a11", "add"), ("a22", "add")]),        # (A11+A22)*(B11+B22)
    ("M2", ("s", 1), [("a21", "add"), ("a22", "sub")]),        # (A21+A22)*B11
    ("M6", ("s", 2), [("a22", "add")]),                        # (A21-A11)*(B11+B12)
]
# phase 5 (chunk stream indices 5 and 6): M5n=(A11+A12)*(-B22) then
# M7=(A12-A22)*(B21+B22) accumulated into the same PSUM bank per chunk.

# S-combos: (op, in0 block, in1 block) in xt layout offsets (bt_off, ko_off)
_SCOMBOS = [
    ("add", (0, 0), (BT2, KO2)),    # S1 = A11+A22   (M1, phase 2)
    ("add", (BT2, 0), (BT2, KO2)),  # S2 = A21+A22   (M2, phase 3)
    ("sub", (BT2, 0), (0, 0)),      # S6 = A21-A11   (M6, phase 4)
    ("add", (0, 0), (0, KO2)),      # S5 = A11+A12   (M5n, phase 5)
    ("sub", (0, KO2), (BT2, KO2)),  # S7 = A12-A22   (M7, phase 5)
]
# S index -> phase that consumes it (build emitted one phase ahead)
_S_USER = {0: 2, 1: 3, 2: 4, 3: 5, 4: 6}

# acc -> (phase finalizing it, stats row-half is top?)
_FINAL_PHASE = {"a21": 3, "a22": 4, "a12": 5, "a11": 5}


def _build(apply_bias: bool, apply_affine: bool):
    key = (apply_bias, apply_affine)
    if key in _BUILD_CACHE:
        return _BUILD_CACHE[key]

    import concourse.mybir as mybir
    import concourse.tile as tile
    from concourse import bacc

    f16 = mybir.dt.float16
    f32 = mybir.dt.float32

    nc = bacc.Bacc("TRN2", target_bir_lowering=False, debug=False,
                   num_devices=NCORES)

    xt_d = nc.declare_dram_parameter("xt", [BT, P, KO, P], f16, isOutput=False)
    # t_d[mi, jc, p, ko, jl] = T_mi[ko*128+p, jc*JC+jl]
    t_d = nc.declare_dram_parameter("t", [7, NJC2, P, KO2, JC], f16,
                                    isOutput=False)
    bias_d = nc.declare_dram_parameter("bias", [DIM], f32, isOutput=False)
    gamma_d = nc.declare_dram_parameter("gamma", [DIM], f32, isOutput=False)
    beta_d = nc.declare_dram_parameter("beta", [DIM], f32, isOutput=False)
    out_d = nc.declare_dram_parameter("out", [BT, P, 2 * NJC2, JC], f16,
                                      isOutput=True)

    with tile.TileContext(nc) as tc:
        with tc.tile_pool(name="xpool", bufs=1) as xpool, \
             tc.tile_pool(name="spool", bufs=2) as spool, \
             tc.tile_pool(name="wpool", bufs=8) as wpool, \
             tc.tile_pool(name="apool", bufs=1) as apool, \
             tc.tile_pool(name="small", bufs=4) as small, \
             tc.tile_pool(name="ppool", bufs=7, space="PSUM") as ppool:

            # --- PE warmup: get the HAM clock gate to 8/8 during the DMA
            # head; more batches are interleaved after the first real groups
            # to bridge DMA-ramp stalls without letting HAM re-throttle.
            warm_sb = small.tile([P, 2 * P], f16, name="warm_sb", tag="warm")
            nc.vector.memset(warm_sb, 0.0)
            warm_ps = ppool.tile([P, JC], f32, name="warm_ps", tag="ps")

            def emit_warm(n):
                for _ in range(n):
                    nc.tensor.matmul(warm_ps[:, 0:P], lhsT=warm_sb[:, 0:P],
                                     rhs=warm_sb[:, P:2 * P],
                                     start=True, stop=True)

            emit_warm(NWARM_HEAD)

            # --- B-combo stream: chunk = [P, KO2, JC] fp16 (1 MiB) streamed
            # as two [P, 8, JC] pieces on two DMA rings; the first chunk is
            # split finer so the first accumulation group starts after
            # ~128 KiB.  Phase 5 interleaves the M5n and M7 streams.
            _CHUNKS = [(mi, jc) for mi in range(5) for jc in range(NJC2)] + \
                      [(mi, jc) for jc in range(NJC2) for mi in (5, 6)]
            w_tiles = {}

            def emit_w_chunk(fi):
                mi, jc = _CHUNKS[fi]
                t0 = wpool.tile([P, 8, JC], f16, name="w_sb", tag="w")
                t1 = wpool.tile([P, 8, JC], f16, name="w_sb", tag="w")
                if fi == 0:
                    for a, b in ((0, 1), (1, 2), (2, 4), (4, 8)):
                        nc.sync.dma_start(out=t0[:, a:b],
                                          in_=t_d[mi, jc, :, a:b])
                else:
                    nc.sync.dma_start(out=t0, in_=t_d[mi, jc, :, 0:8])
                nc.scalar.dma_start(out=t1, in_=t_d[mi, jc, :, 8:16])
                w_tiles[fi] = (t0, t1)

            NPREF = 4
            for fi0 in range(NPREF):
                emit_w_chunk(fi0)

            # --- x stream: ALL on the gpsimd ring (sync carries w-t0,
            # scalar carries w-t1; a third stream on either would delay
            # w pieces).  Emission order == need order: A11 per b-tile
            # (phase 0 consumes btl 0..3 in sequence), then A22 (phase 1),
            # then A21/A12 (S-combos of phases 3+).
            xt_sb = xpool.tile([P, BT, KO, P], f16)

            def emit_x(bt, a, b):
                nc.gpsimd.dma_start(out=xt_sb[:, bt, a:b],
                                    in_=xt_d[bt, :, a:b])

            emit_x(0, 0, 2)
            emit_x(0, 2, 4)
            emit_x(0, 4, 8)
            emit_x(0, 8, 16)
            # bt2's A11 rides the scalar ring (slotted after w-t1 chunks
            # 0-1, ahead of 2+): the gpsimd ring alone can't deliver all of
            # A11 at the rate the first jc pass consumes it.
            for bt in (1, 3):
                emit_x(bt, 0, 8)
                emit_x(bt, 8, 16)
            nc.scalar.dma_start(out=xt_sb[:, 2, 0:16], in_=xt_d[2, :, 0:16])
            for bt in range(BT2, BT):          # A22
                emit_x(bt, KO2, KO2 + 8)
                emit_x(bt, KO2 + 8, KO)
            for bt in range(BT2, BT):          # A21
                emit_x(bt, 0, 16)
            for bt in range(0, BT2):           # A12
                emit_x(bt, KO2, KO)

            # --- S-combo tiles (2 rotating slots; built one phase ahead).
            s_tiles = {}

            def emit_s(si):
                op, (b0, k0), (b1, k1) = _SCOMBOS[si]
                s_sb = spool.tile([P, BT2, KO2, P], f16, name="s_sb", tag="s")
                fn = nc.vector.tensor_add if op == "add" else nc.vector.tensor_sub
                for btl in range(BT2):
                    fn(s_sb[:, btl],
                       xt_sb[:, b0 + btl, k0:k0 + KO2],
                       xt_sb[:, b1 + btl, k1:k1 + KO2])
                s_tiles[si] = s_sb

            emit_s(0)  # S1: build under the two raw-A multiplies

            # --- quadrant accumulators (fp16) + LayerNorm state
            acc = {q: apool.tile([P, BT2, NJC2, JC], f16, name=q, tag=q)
                   for q in ("a11", "a12", "a21", "a22")}
            stats_sb = spool.tile([P, BT, 2 * NJC2, 6], f32, name="stats",
                                  tag="stats")
            eps_sb = small.tile([P, 1], f32, name="eps", tag="eps")
            nc.vector.memset(eps_sb, EPS)

            bias_sb = None
            if apply_bias:
                bias_sb = spool.tile([P, 2 * NJC2, JC], f32, name="bias_sb",
                                     tag="bias")
                nc.sync.dma_start(out=bias_sb,
                                  in_=bias_d.ap().to_broadcast(
                                      [P, 2 * NJC2, JC]))
            gamma_sb = beta_sb = None
            if apply_affine:
                gamma_sb = spool.tile([P, 2 * NJC2, JC], f32, name="gamma_sb",
                                      tag="gamma")
                nc.sync.dma_start(out=gamma_sb,
                                  in_=gamma_d.ap().to_broadcast(
                                      [P, 2 * NJC2, JC]))
                beta_sb = spool.tile([P, 2 * NJC2, JC], f32, name="beta_sb",
                                     tag="beta")
                nc.sync.dma_start(out=beta_sb,
                                  in_=beta_d.ap().to_broadcast(
                                      [P, 2 * NJC2, JC]))

            def finalize_chunk(dst, bt, ci):
                if apply_bias:
                    nc.vector.tensor_add(dst, dst, bias_sb[:, ci])
                nc.vector.bn_stats(stats_sb[:, bt, ci], dst)

            def layernorm_apply(bt):
                """Normalize row-block bt (128 rows x 4096) and DMA it out.

                Rows are partitions, so mean/rstd are per-partition scalars.
                Left half applies on DVE (tensor_scalar, 2x fp16 mode),
                right half on ACT (Identity with per-partition scale/bias),
                so the two halves run concurrently.
                """
                top = bt < BT2
                btl = bt if top else bt - BT2
                accL = acc["a11"] if top else acc["a21"]
                accR = acc["a12"] if top else acc["a22"]
                mv = small.tile([P, 2], f32, name="mv", tag="mv")
                nc.vector.bn_aggr(mv, stats_sb[:, bt])
                std = small.tile([P, 1], f32, name="std", tag="std")
                nc.scalar.activation(std, mv[:, 1:2],
                                     mybir.ActivationFunctionType.Sqrt,
                                     bias=eps_sb)
                rstd = small.tile([P, 1], f32, name="rstd", tag="rstd")
                nc.vector.reciprocal(rstd, std)
                nmr = small.tile([P, 1], f32, name="nmr", tag="nmr")
                nc.vector.tensor_scalar(
                    nmr, mv[:, 0:1], scalar1=rstd, scalar2=-1.0,
                    op0=mybir.AluOpType.mult, op1=mybir.AluOpType.mult,
                )
                # normalize in place (the acc chunks are dead after this)
                # and DMA straight out of the accumulators -- no staging
                # tile, so nothing serializes on an output-buffer slot.
                oL = accL[:, btl]
                nc.vector.tensor_scalar(
                    oL, oL, scalar1=mv[:, 0:1], scalar2=rstd,
                    op0=mybir.AluOpType.subtract, op1=mybir.AluOpType.mult,
                )
                # right half splits across ACT and DVE so the per-row-block
                # apply critical path is ~1.3us instead of ~2.1us.
                oR = accR[:, btl]
                h = NJC2 // 2
                nc.scalar.activation(
                    oR[:, 0:h], oR[:, 0:h],
                    mybir.ActivationFunctionType.Identity,
                    bias=nmr, scale=rstd,
                )
                nc.vector.tensor_scalar(
                    oR[:, h:], oR[:, h:], scalar1=mv[:, 0:1], scalar2=rstd,
                    op0=mybir.AluOpType.subtract, op1=mybir.AluOpType.mult,
                )
                for side, o in ((0, oL), (1, oR)):
                    if apply_affine:
                        g = gamma_sb[:, side * NJC2:(side + 1) * NJC2]
                        bta = beta_sb[:, side * NJC2:(side + 1) * NJC2]
                        nc.vector.tensor_mul(o, o, g)
                        nc.vector.tensor_add(o, o, bta)
                    nc.gpsimd.dma_start(
                        out=out_d[bt, :, side * NJC2:(side + 1) * NJC2],
                        in_=o)

            def mm_group(ps, lhsT, fi, start, stop, skip_check=False):
                w0, w1 = w_tiles[fi]
                for ko in range(KO2):
                    nc.tensor.matmul(
                        ps,
                        lhsT=lhsT(ko),
                        rhs=(w0 if ko < 8 else w1)[:, ko % 8],
                        start=start and (ko == 0),
                        stop=stop and (ko == KO2 - 1),
                        skip_group_check=skip_check,
                    )

            # --- phases 0-4
            fi = 0
            for mi, (mname, lhs, dests) in enumerate(_MULS):
                for si, user in _S_USER.items():
                    if user == mi + 1:
                        emit_s(si)

                if lhs[0] == "x":
                    _, b_off, k_off = lhs
                    def lhsT2(btl, ko, b_off=b_off, k_off=k_off):
                        return xt_sb[:, b_off + btl, k_off + ko]
                else:
                    s_sb = s_tiles.pop(lhs[1])
                    def lhsT2(btl, ko, s_sb=s_sb):
                        return s_sb[:, btl, ko]

                for jc in range(NJC2):
                    if fi + NPREF < len(_CHUNKS):
                        emit_w_chunk(fi + NPREF)
                    for btl in range(BT2):
                        ps = ppool.tile([P, JC], f32, name="ps", tag="ps")
                        mm_group(ps, lambda ko, btl=btl: lhsT2(btl, ko),
                                 fi, True, True)
                        # bridge per-group x/w-arrival stalls while the DMA
                        # rings ramp (first jc pass only)
                        if mi == 0 and jc == 0 and btl < 3:
                            emit_warm(8)
                        for acc_name, mode in dests:
                            a_t = acc[acc_name]
                            dst = a_t[:, btl, jc]
                            if mode == "copy":
                                nc.scalar.activation(
                                    dst, ps,
                                    mybir.ActivationFunctionType.Copy)
                            elif mode == "add":
                                nc.vector.tensor_add(dst, dst, ps)
                            else:
                                nc.vector.tensor_sub(dst, dst, ps)
                            if _FINAL_PHASE[acc_name] == mi:
                                bt = btl if acc_name in ("a11", "a12") \
                                    else BT2 + btl
                                right = acc_name in ("a12", "a22")
                                ci = (NJC2 + jc) if right else jc
                                finalize_chunk(dst, bt, ci)
                    w_tiles.pop(fi)
                    # keep the PE fed through the DMA-ramp head
                    if mi == 0 and 1 <= jc <= 3:
                        emit_warm(8)
                    if mname == "M6" and jc == NJC2 - 1:
                        # DVE FIFO order matters: the last jc pass's RMWs
                        # went first (PSUM banks recycle promptly), then the
                        # S7 build, then the bottom applies -- so phase 5's
                        # first M7 group waits only on the S7 build, and the
                        # applies (with a whole phase of slack) come last.
                        emit_s(4)
                        for bt in range(BT2, BT):
                            layernorm_apply(bt)
                    fi += 1

            # --- phase 5: M5n and M7 share one PSUM bank per (jc, btl).
            # Emission is software-pipelined (M5n btl+1 between M5n btl's
            # RMW and M7 btl) so the PE never waits on the mid-bank read.
            s5 = s_tiles.pop(3)
            s7 = s_tiles.pop(4)
            for jc in range(NJC2):
                f5 = fi + 2 * jc          # M5n chunk
                f7 = f5 + 1               # M7 chunk
                for df in (NPREF, NPREF + 1):
                    if f5 + df < len(_CHUNKS):
                        emit_w_chunk(f5 + df)
                ps_t = {}
                for k in range(BT2 + 1):
                    if k < BT2:
                        btl = k
                        ps = ppool.tile([P, JC], f32, name="ps", tag="ps")
                        ps_t[btl] = ps
                        mm_group(ps, lambda ko, btl=btl: s5[:, btl, ko],
                                 f5, True, True)
                        dst = acc["a12"][:, btl, jc]
                        nc.vector.tensor_sub(dst, dst, ps)
                        finalize_chunk(dst, btl, NJC2 + jc)
                    if k >= 1:
                        btl = k - 1
                        ps = ps_t.pop(btl)
                        # accumulate M7 on top of M5n (has_written bits stay
                        # set from the completed M5n group -> PE adds).
                        mm_group(ps, lambda ko, btl=btl: s7[:, btl, ko],
                                 f7, False, True, skip_check=True)
                        dst = acc["a11"][:, btl, jc]
                        nc.vector.tensor_add(dst, dst, ps)
                        finalize_chunk(dst, btl, jc)
                        if jc == NJC2 - 1:
                            # top row-block final: applies stagger per btl
                            # through the last jc pass.
                            layernorm_apply(btl)
                w_tiles.pop(f5)
                w_tiles.pop(f7)

    nc.compile()
    _BUILD_CACHE[key] = nc
    return nc


def kernel(x, W_qkv, b_qkv, W_proj, b_proj, gamma, beta):
    from concourse.bass_utils import run_bass_kernel_spmd

    x = np.asarray(x, dtype=np.float32)
    W_qkv = np.asarray(W_qkv, dtype=np.float32)
    b_qkv = np.asarray(b_qkv, dtype=np.float32)
    W_proj = np.asarray(W_proj, dtype=np.float32)
    b_proj = np.asarray(b_proj, dtype=np.float32)
    gamma = np.asarray(gamma, dtype=np.float32)
    beta = np.asarray(beta, dtype=np.float32)

    # Fold the two projections (q/k are dead: seq len 1 => attention == v).
    W_v = W_qkv[2 * DIM:3 * DIM, :]
    Bm = np.ascontiguousarray((W_proj @ W_v).T)   # [k, n]
    bias_total = W_proj @ b_qkv[2 * DIM:] + b_proj

    B11 = Bm[:HN, :HN]
    B12 = Bm[:HN, HN:]
    B21 = Bm[HN:, :HN]
    B22 = Bm[HN:, HN:]
    # B-combos in phase order: M3, M4, M1, M2, M6, M5n, M7
    Ts = [B12 - B22, B21 - B11, B11 + B22, B11, B11 + B12, -B22, B21 + B22]

    def tile_t(T):
        # [jc, p, ko, jl] = T[ko*128+p, jc*JC+jl]
        return np.ascontiguousarray(
            T.reshape(KO2, P, NJC2, JC).transpose(2, 1, 0, 3)
        ).astype(np.float16)

    t_host = np.stack([tile_t(T) for T in Ts])

    apply_bias = bool(np.any(bias_total))
    apply_affine = not (np.all(gamma == 1.0) and np.all(beta == 0.0))

    nc = _build(apply_bias, apply_affine)

    in_maps = []
    for i in range(NCORES):
        xs = x[i * BL:(i + 1) * BL]           # [BL, DIM]
        # xt[bt, p, ko, b'] = xs[bt*P + b', ko*P + p]
        xt = np.ascontiguousarray(
            xs.T.reshape(KO, P, BT, P).transpose(2, 1, 0, 3)
        ).astype(np.float16)
        in_maps.append({
            "xt": xt,
            "t": t_host,
            "bias": bias_total,
            "gamma": gamma,
            "beta": beta,
        })

    trace = bool(int(os.environ.get("KERNEL_TRACE", "0")))
    res = run_bass_kernel_spmd(nc, in_maps, core_ids=list(range(NCORES)),
                               trace=trace)
    if trace:
        kernel.last_exec_time_ns = res.exec_time_ns
        kernel.last_results = res

    out = np.concatenate(
        [r["out"].reshape(BL, DIM).astype(np.float32) for r in res.results],
        axis=0,
    )
    return out
